# revision 11
# baseline (speedup 1.0000x reference)
"""Trainium2 Bass kernel for nn_JunctionCountsModel (gnn_message_passing).

out[n] = softplus(gelu(x[b,d] @ Wd + x[b,a] @ Wa + feat[n] @ Wc + b_in) @ Wo + b_out)

Sharding: data-parallel over the N=200000 junctions across 8 NeuronCores
(25000 each, padded to 25088 = 49*512); x and the MLP weights replicated.

The runtime in this container rejects the SWDGE custom-ucode DMA ops
(dma_gather / dma_scatter_add fail with INTERNAL at NEFF execution), and the
surviving gather primitives are too slow for 50k rows/core (indirect_dma_start
~1.5us/128 rows of Pool-SEQ overhead; ap_gather scans its whole SBUF table per
call).  So the donor/acceptor row gather runs on the host (numpy fancy
indexing over the known index tensor, exactly the prep step that previously
built the dma_gather index tables), staged per core as a [128, 2, NPCP] bf16
tensor in (K-partitions, donor|acceptor, junction) layout — the exact operand
layout the tensor engine wants, so the device runs a pure streaming MLP.

Device-side structure per 512-junction tile:
  - input layer: per 3-h-block group, 9 matmuls accumulate
    Wd^T xd + Wa^T xa + Wc_ext^T ft into a 3-bank PSUM tile; Wc_ext carries
    [W_distembed; w_dist; b_in] with ft rows [thresholds; dist/L; ones], so
    the bias is part of the matmul and one merged gelu activation covers all
    3 h-blocks (fewer, larger ACT instructions).
  - output layer: junction-major — z^T[128j, 32] = sum_hb yg[hb]^T @ Wo[hb],
    4 sub-tiles per tile accumulated in a single PSUM bank, copied into an
    SBUF-resident z buffer.  No DRAM round-trip for z.
  - phase 2 (after all gelus, so the ACT table swaps exactly once):
    out = ln(1 + exp(z)) over [128, big] slabs, DMA'd out in the blocked
    layout; the host unshuffles.
"""

import os
import sys

sys.path.insert(0, "/opt/trn_rl_repo")

import ml_dtypes
import numpy as np

B, L, K = 4, 32768, 128
N, H, D, T = 200000, 768, 15, 32
NCORES = 8
NPC = N // NCORES  # junctions per core
NPCP = 25088  # padded to 49 * 512
HB = H // 128  # h-blocks
HG = 2  # activation groups of 3 h-blocks
TILE = 512
NT = NPCP // TILE  # 49 tiles
CHUNK = 2048  # junctions per input DMA chunk
ZCOLS = NPCP // TILE * 128  # z buffer free cols per partition (49*128=6272)
FCH = 4  # phase-2 chunks

BF16 = ml_dtypes.bfloat16

# Results of the most recent device run (for test harness introspection).
LAST_RESULTS = None
USED_FALLBACK = False
_BUILD_CACHE = {}


def _build():
    """Build the per-core Bass program (identical across the 8 cores)."""
    if "nc" in _BUILD_CACHE:
        return _BUILD_CACHE["nc"]
    import bass_rust
    import concourse.bacc as bacc
    import concourse.mybir as mybir
    import concourse.tile as tile

    dt = mybir.dt
    AF = mybir.ActivationFunctionType
    gelu_fn = AF.Tanh if os.environ.get("KERNEL_SIM_ACT") else AF.Gelu_apprx_tanh

    nc = bacc.Bacc("TRN2", target_bir_lowering=False, debug=False)
    xin = nc.dram_tensor("xin", [128, 3, NPCP], dt.bfloat16, kind="ExternalInput")
    wd = nc.dram_tensor("wd", [K, H], dt.bfloat16, kind="ExternalInput")
    wa = nc.dram_tensor("wa", [K, H], dt.bfloat16, kind="ExternalInput")
    wc = nc.dram_tensor("wc", [K, H], dt.bfloat16, kind="ExternalInput")
    wo = nc.dram_tensor("wo", [128, HB, T], dt.bfloat16, kind="ExternalInput")
    outb = nc.dram_tensor("outb", [128, ZCOLS], dt.float32, kind="ExternalOutput")

    with tile.TileContext(nc) as tc:
        with (
            tc.tile_pool(name="const", bufs=1) as const,
            tc.tile_pool(name="xp", bufs=3) as xp,
            tc.tile_pool(name="ygp", bufs=2) as ygp,
            tc.tile_pool(name="finp", bufs=4) as finp,
            tc.tile_pool(name="ypsum", bufs=2, space="PSUM") as ypsum,
            tc.tile_pool(name="opsum", bufs=2, space="PSUM") as opsum,
        ):
            wd_sb = const.tile([K, H], dt.bfloat16, tag="wd")
            wa_sb = const.tile([K, H], dt.bfloat16, tag="wa")
            wc_sb = const.tile([K, H], dt.bfloat16, tag="wc")
            wo_sb = const.tile([128, HB, T], dt.bfloat16, tag="wo")
            zbuf = const.tile([128, ZCOLS], dt.float32, tag="zbuf")
            nc.sync.dma_start(out=wd_sb[:], in_=wd[:])
            nc.sync.dma_start(out=wa_sb[:], in_=wa[:])
            nc.sync.dma_start(out=wc_sb[:], in_=wc[:])
            nc.sync.dma_start(out=wo_sb[:], in_=wo[:])

            def emit_output(yg, ti):
                """Output layer for a finished tile: z^T[128j, T] into zbuf."""
                zps = opsum.tile([128, 4, T], dt.float32, tag="zps")
                for s in range(4):
                    for hb in range(HB):
                        nc.tensor.matmul(
                            zps[:, s, :],
                            yg[:, hb, s * 128 : (s + 1) * 128],
                            wo_sb[:, hb, :],
                            start=(hb == 0), stop=(hb == HB - 1),
                        )
                nc.vector.tensor_copy(zbuf[:, ti * 128 : (ti + 1) * 128], zps[:])

            # Software-pipelined main loop: tile t's output matmuls are
            # emitted after tile t+1's input matmuls, so the tensor engine
            # never waits on the gelu and stays continuously busy (p-state
            # ramps to full clock).
            last_act = None
            pending = None  # (yg, tile_index) awaiting its output layer
            # first chunk is a single tile so the tensor engine starts ~3us
            # earlier (not gated on a full 1.5MB chunk DMA)
            bounds = [0, TILE]
            while bounds[-1] < NPCP:
                bounds.append(min(bounds[-1] + CHUNK, NPCP))
            for c0, c1 in zip(bounds, bounds[1:]):
                ch = c1 - c0
                xt = xp.tile([128, 3, CHUNK], dt.bfloat16, tag="xt")
                nc.sync.dma_start(out=xt[:, :, :ch], in_=xin[:, :, c0 : c0 + ch])
                for t0 in range(0, ch, TILE):
                    j0 = c0 + t0  # global junction column of this tile
                    yg = ygp.tile([128, HB, TILE], dt.bfloat16, tag="yg")
                    for g in range(HG):
                        yps = ypsum.tile([128, 3, TILE], dt.float32, tag="yps")
                        for i in range(3):
                            hb = 3 * g + i
                            hsl = slice(hb * 128, (hb + 1) * 128)
                            nc.tensor.matmul(
                                yps[:, i, :], wd_sb[:, hsl],
                                xt[:, 0, t0 : t0 + TILE],
                                start=True, stop=False,
                            )
                            nc.tensor.matmul(
                                yps[:, i, :], wa_sb[:, hsl],
                                xt[:, 1, t0 : t0 + TILE],
                                start=False, stop=False,
                            )
                            nc.tensor.matmul(
                                yps[:, i, :], wc_sb[:, hsl],
                                xt[:, 2, t0 : t0 + TILE],
                                start=False, stop=True,
                            )
                        last_act = nc.scalar.activation(
                            yg[:, 3 * g : 3 * g + 3, :], yps[:], gelu_fn
                        )
                    if pending is not None:
                        emit_output(*pending)
                    pending = (yg, j0 // TILE)
            emit_output(*pending)

            # phase 2: out = ln(1 + exp(z)); gated behind the last gelu so the
            # ACT table swaps exactly once
            AFexp = mybir.ActivationFunctionType.Exp
            AFln = mybir.ActivationFunctionType.Ln
            fch = ZCOLS // FCH
            # all Exp chunks first, then all Ln chunks: exactly two ACT table
            # loads (the greedy table picker would otherwise thrash
            # exp_and_others <-> natural_log per chunk)
            ebuf = const.tile([128, ZCOLS], dt.float32, tag="ebuf")
            last_exp = nc.scalar.activation(ebuf[:], zbuf[:], AFexp)
            bass_rust.add_dep_helper(
                last_exp.ins, last_act.ins, True,
                "exp phase strictly after all gelus",
            )
            for q in range(FCH):
                sl = slice(q * fch, (q + 1) * fch)
                ot = finp.tile([128, fch], dt.float32, tag="ot")
                ol = nc.scalar.activation(ot[:], ebuf[:, sl], AFln, bias=1.0)
                bass_rust.add_dep_helper(
                    ol.ins, last_exp.ins, True,
                    "ln phase strictly after all exps",
                )
                nc.sync.dma_start(out=outb[:, sl], in_=ot[:])
    nc.compile()
    _BUILD_CACHE["nc"] = nc
    return nc


def _prep_inputs(x, xxj, W_donor, W_acceptor, w_dist, W_distembed, b_in, W_out, b_out):
    """Host prep: gather donor/acceptor rows, build features, pack weights."""
    xflat = np.asarray(x, np.float32).reshape(B * L, K)
    bi = xxj[:, 0].astype(np.int64)
    di = xxj[:, 1].astype(np.int64)
    ai = xxj[:, 2].astype(np.int64)
    dist = np.abs(ai - di)

    wc_ext = np.zeros((K, H), np.float32)
    wc_ext[:D] = np.asarray(W_distembed, np.float32)
    wc_ext[D] = np.asarray(w_dist, np.float32)
    wc_ext[D + 1] = np.asarray(b_in, np.float32)
    wmap = {
        "wd": np.asarray(W_donor).astype(BF16),
        "wa": np.asarray(W_acceptor).astype(BF16),
        "wc": wc_ext.astype(BF16),
        "wo": np.ascontiguousarray(
            np.asarray(W_out).astype(BF16).reshape(HB, 128, T).transpose(1, 0, 2)
        ),
    }

    in_maps = []
    for c in range(NCORES):
        sl = slice(c * NPC, (c + 1) * NPC)
        xin = np.zeros((128, 3, NPCP), BF16)
        rows_d = bi[sl] * L + di[sl]
        rows_a = bi[sl] * L + ai[sl]
        xin[:, 0, :NPC] = xflat[rows_d].astype(BF16).T
        xin[:, 1, :NPC] = xflat[rows_a].astype(BF16).T
        dc = dist[sl]
        ft = np.zeros((D + 2, NPC), np.float32)
        for dd in range(D):
            ft[dd] = dc >= (1 << dd)
        ft[D] = dc.astype(np.float32) / L
        ft[D + 1] = 1.0  # carries b_in through wc_ext
        xin[: D + 2, 2, :NPC] = ft.astype(BF16)
        in_maps.append({"xin": xin, **wmap})
    return in_maps


def _numpy_fallback(
    x, xxj, W_donor, W_acceptor, w_dist, W_distembed, b_in, W_out, b_out
):
    bi, di, ai = xxj[:, 0], xxj[:, 1], xxj[:, 2]
    out = np.empty((N, T), np.float32)
    for s0 in range(0, N, 20000):
        s = slice(s0, min(s0 + 20000, N))
        xd = x[bi[s], di[s], :]
        xa = x[bi[s], ai[s], :]
        dist = np.abs(ai[s].astype(np.int64) - di[s].astype(np.int64))
        de = (dist[:, None] >= (1 << np.arange(D))[None, :]).astype(np.float32)
        dn = dist.astype(np.float32) / L
        y = (xd @ W_donor + xa @ W_acceptor + dn[:, None] * w_dist[None, :]
             + de @ W_distembed + b_in[None, :])
        c = np.float32(np.sqrt(2.0 / np.pi))
        y = 0.5 * y * (1.0 + np.tanh(c * (y + np.float32(0.044715) * y ** 3)))
        z = y @ W_out + b_out[None, :]
        out[s] = np.log1p(np.exp(-np.abs(z))) + np.maximum(z, 0.0)
    return out


def kernel(
    x, xxj_sparse, W_donor, W_acceptor, w_dist, W_distembed, b_in, W_out, b_out
):
    global LAST_RESULTS, USED_FALLBACK
    from concourse.bass_utils import run_bass_kernel_spmd

    x = np.asarray(x)
    xxj = np.asarray(xxj_sparse)
    if np.any(np.asarray(b_out)):
        print("kernel: nonzero b_out not folded on device; using numpy path")
        USED_FALLBACK = True
        return _numpy_fallback(
            x, xxj, W_donor, W_acceptor, w_dist, W_distembed, b_in, W_out, b_out
        )
    in_maps = _prep_inputs(
        x, xxj, W_donor, W_acceptor, w_dist, W_distembed, b_in, W_out, b_out
    )

    try:
        nc = _build()
        res = run_bass_kernel_spmd(
            nc, in_maps, core_ids=list(range(NCORES)),
            trace=bool(int(os.environ.get("KERNEL_TRACE", "0"))),
        )
    except Exception:
        import traceback

        traceback.print_exc()
        USED_FALLBACK = True
        return _numpy_fallback(
            x, xxj, W_donor, W_acceptor, w_dist, W_distembed, b_in, W_out, b_out
        )
    LAST_RESULTS = res
    USED_FALLBACK = False

    out = np.empty((N, T), np.float32)
    for c, r in enumerate(res.results):
        ob = r["outb"]  # [128, ZCOLS] = [128, NT, 4, 32] blocked
        core = ob.reshape(128, NT, 4, T).transpose(1, 2, 0, 3).reshape(NPCP, T)
        out[c * NPC : (c + 1) * NPC] = core[:NPC]
    return out


# revision 14
# speedup vs baseline: 1.1717x; 1.1717x over previous
"""Trainium2 Bass kernel for nn_JunctionCountsModel (gnn_message_passing).

out[n] = softplus(gelu(x[b,d] @ Wd + x[b,a] @ Wa + feat[n] @ Wc + b_in) @ Wo + b_out)

Sharding: data-parallel over the N=200000 junctions across 8 NeuronCores
(25000 each, padded to 25088 = 49*512); x and the MLP weights replicated.

The runtime in this container rejects the SWDGE custom-ucode DMA ops
(dma_gather / dma_scatter_add fail with INTERNAL at NEFF execution), and the
surviving gather primitives are too slow for 50k rows/core (indirect_dma_start
~1.5us/128 rows of Pool-SEQ overhead; ap_gather scans its whole SBUF table per
call).  So the donor/acceptor row gather runs on the host (numpy fancy
indexing over the known index tensor, exactly the prep step that previously
built the dma_gather index tables), staged per core as a [128, 3, NPCP] bf16
tensor in (K-partitions, donor|acceptor|feat, junction) layout — the exact
operand layout the tensor engine wants, so the device runs a pure streaming
MLP.

Device-side structure per 512-junction tile:
  - input layer: per 3-h-block group, 9 matmuls accumulate
    Wd^T xd + Wa^T xa + Wc_ext^T ft into a 3-bank PSUM tile; Wc_ext carries
    [W_distembed; w_dist; b_in] with ft rows [thresholds; dist/L; ones], so
    the bias is part of the matmul and one merged gelu activation covers all
    3 h-blocks (fewer, larger ACT instructions).  The feat operand is
    ZERO-PADDED to contract=128: a contract-17 matmul starves the PE
    activity monitor (HAM) and locks the whole stream at the cold 1.2 GHz
    clock (measured 2.07x slowdown); full-height operands keep it at 2.4.
  - output layer: junction-major — z^T[128j, 32] = sum_hb yg[hb]^T @ Wo[hb],
    4 sub-tiles per tile accumulated in a single PSUM bank, copied into an
    SBUF-resident z buffer.  No DRAM round-trip for z.
  - phase 2 (after all gelus, so the ACT table swaps exactly once):
    out = ln(1 + exp(z)) over [128, big] slabs, DMA'd out in the blocked
    layout; the host unshuffles.
"""

import os
import sys

sys.path.insert(0, "/opt/trn_rl_repo")

import ml_dtypes
import numpy as np

B, L, K = 4, 32768, 128
N, H, D, T = 200000, 768, 15, 32
NCORES = 8
NPC = N // NCORES  # junctions per core
NPCP = 25088  # padded to 49 * 512
HB = H // 128  # h-blocks
HG = 2  # activation groups of 3 h-blocks
TILE = 512
NT = NPCP // TILE  # 49 tiles
CHUNK = 2048  # junctions per input DMA chunk
ZCOLS = NPCP // TILE * 128  # z buffer free cols per partition (49*128=6272)
FCH = 4  # phase-2 chunks

BF16 = ml_dtypes.bfloat16

# Results of the most recent device run (for test harness introspection).
LAST_RESULTS = None
USED_FALLBACK = False
_BUILD_CACHE = {}


def _build():
    """Build the per-core Bass program (identical across the 8 cores)."""
    if "nc" in _BUILD_CACHE:
        return _BUILD_CACHE["nc"]
    import bass_rust
    import concourse.bacc as bacc
    import concourse.mybir as mybir
    import concourse.tile as tile

    dt = mybir.dt
    AF = mybir.ActivationFunctionType
    gelu_fn = AF.Tanh if os.environ.get("KERNEL_SIM_ACT") else AF.Gelu_apprx_tanh

    nc = bacc.Bacc("TRN2", target_bir_lowering=False, debug=False)
    xin = nc.dram_tensor("xin", [128, 3, NPCP], dt.bfloat16, kind="ExternalInput")
    wd = nc.dram_tensor("wd", [K, H], dt.bfloat16, kind="ExternalInput")
    wa = nc.dram_tensor("wa", [K, H], dt.bfloat16, kind="ExternalInput")
    wc = nc.dram_tensor("wc", [K, H], dt.bfloat16, kind="ExternalInput")
    wo = nc.dram_tensor("wo", [128, HB, T], dt.bfloat16, kind="ExternalInput")
    outb = nc.dram_tensor("outb", [128, ZCOLS], dt.float32, kind="ExternalOutput")

    with tile.TileContext(nc) as tc:
        with (
            tc.tile_pool(name="const", bufs=1) as const,
            tc.tile_pool(name="xp", bufs=3) as xp,
            tc.tile_pool(name="ygp", bufs=2) as ygp,
            tc.tile_pool(name="finp", bufs=4) as finp,
            tc.tile_pool(name="ypsum", bufs=2, space="PSUM") as ypsum,
            tc.tile_pool(name="opsum", bufs=2, space="PSUM") as opsum,
        ):
            wd_sb = const.tile([K, H], dt.bfloat16, tag="wd")
            wa_sb = const.tile([K, H], dt.bfloat16, tag="wa")
            wc_sb = const.tile([K, H], dt.bfloat16, tag="wc")
            wo_sb = const.tile([128, HB, T], dt.bfloat16, tag="wo")
            zbuf = const.tile([128, ZCOLS], dt.float32, tag="zbuf")
            nc.sync.dma_start(out=wd_sb[:], in_=wd[:])
            nc.sync.dma_start(out=wa_sb[:], in_=wa[:])
            nc.sync.dma_start(out=wc_sb[:], in_=wc[:])
            nc.sync.dma_start(out=wo_sb[:], in_=wo[:])

            def emit_output(yg, ti):
                """Output layer for a finished tile: z^T[128j, T] into zbuf."""
                zps = opsum.tile([128, 4, T], dt.float32, tag="zps")
                for s in range(4):
                    for hb in range(HB):
                        nc.tensor.matmul(
                            zps[:, s, :],
                            yg[:, hb, s * 128 : (s + 1) * 128],
                            wo_sb[:, hb, :],
                            start=(hb == 0), stop=(hb == HB - 1),
                        )
                nc.vector.tensor_copy(zbuf[:, ti * 128 : (ti + 1) * 128], zps[:])

            # Software-pipelined main loop: tile t's output matmuls are
            # emitted after tile t+1's input matmuls, so the tensor engine
            # never waits on the gelu and stays continuously busy (p-state
            # ramps to full clock).
            last_act = None
            pending = None  # (yg, tile_index) awaiting its output layer
            for c0 in range(0, NPCP, CHUNK):
                ch = min(CHUNK, NPCP - c0)
                xt = xp.tile([128, 3, CHUNK], dt.bfloat16, tag="xt")
                nc.sync.dma_start(out=xt[:, :, :ch], in_=xin[:, :, c0 : c0 + ch])
                for t0 in range(0, ch, TILE):
                    j0 = c0 + t0  # global junction column of this tile
                    yg = ygp.tile([128, HB, TILE], dt.bfloat16, tag="yg")
                    for g in range(HG):
                        yps = ypsum.tile([128, 3, TILE], dt.float32, tag="yps")
                        for i in range(3):
                            hb = 3 * g + i
                            hsl = slice(hb * 128, (hb + 1) * 128)
                            nc.tensor.matmul(
                                yps[:, i, :], wd_sb[:, hsl],
                                xt[:, 0, t0 : t0 + TILE],
                                start=True, stop=False,
                            )
                            nc.tensor.matmul(
                                yps[:, i, :], wa_sb[:, hsl],
                                xt[:, 1, t0 : t0 + TILE],
                                start=False, stop=False,
                            )
                            nc.tensor.matmul(
                                yps[:, i, :], wc_sb[:, hsl],
                                xt[:, 2, t0 : t0 + TILE],
                                start=False, stop=True,
                            )
                        last_act = nc.scalar.activation(
                            yg[:, 3 * g : 3 * g + 3, :], yps[:], gelu_fn
                        )
                    if pending is not None:
                        emit_output(*pending)
                    pending = (yg, j0 // TILE)
            emit_output(*pending)

            # phase 2: out = ln(1 + exp(z)); gated behind the last gelu so the
            # ACT table swaps exactly once
            AFexp = mybir.ActivationFunctionType.Exp
            AFln = mybir.ActivationFunctionType.Ln
            fch = ZCOLS // FCH
            # all Exp chunks first, then all Ln chunks: exactly two ACT table
            # loads (the greedy table picker would otherwise thrash
            # exp_and_others <-> natural_log per chunk)
            ebuf = const.tile([128, ZCOLS], dt.float32, tag="ebuf")
            last_exp = None
            for q in range(FCH):
                sl = slice(q * fch, (q + 1) * fch)
                ea = nc.scalar.activation(ebuf[:, sl], zbuf[:, sl], AFexp)
                bass_rust.add_dep_helper(
                    ea.ins, last_act.ins, True,
                    "exp phase strictly after all gelus",
                )
                last_exp = ea
            for q in range(FCH):
                sl = slice(q * fch, (q + 1) * fch)
                ot = finp.tile([128, fch], dt.float32, tag="ot")
                ol = nc.scalar.activation(ot[:], ebuf[:, sl], AFln, bias=1.0)
                bass_rust.add_dep_helper(
                    ol.ins, last_exp.ins, True,
                    "ln phase strictly after all exps",
                )
                nc.sync.dma_start(out=outb[:, sl], in_=ot[:])
    nc.compile()
    _BUILD_CACHE["nc"] = nc
    return nc


def _prep_inputs(x, xxj, W_donor, W_acceptor, w_dist, W_distembed, b_in, W_out, b_out):
    """Host prep: gather donor/acceptor rows, build features, pack weights."""
    xflat = np.asarray(x, np.float32).reshape(B * L, K)
    bi = xxj[:, 0].astype(np.int64)
    di = xxj[:, 1].astype(np.int64)
    ai = xxj[:, 2].astype(np.int64)
    dist = np.abs(ai - di)

    wc_ext = np.zeros((K, H), np.float32)
    wc_ext[:D] = np.asarray(W_distembed, np.float32)
    wc_ext[D] = np.asarray(w_dist, np.float32)
    wc_ext[D + 1] = np.asarray(b_in, np.float32)
    wmap = {
        "wd": np.asarray(W_donor).astype(BF16),
        "wa": np.asarray(W_acceptor).astype(BF16),
        "wc": wc_ext.astype(BF16),
        "wo": np.ascontiguousarray(
            np.asarray(W_out).astype(BF16).reshape(HB, 128, T).transpose(1, 0, 2)
        ),
    }

    in_maps = []
    for c in range(NCORES):
        sl = slice(c * NPC, (c + 1) * NPC)
        xin = np.zeros((128, 3, NPCP), BF16)
        rows_d = bi[sl] * L + di[sl]
        rows_a = bi[sl] * L + ai[sl]
        xin[:, 0, :NPC] = xflat[rows_d].astype(BF16).T
        xin[:, 1, :NPC] = xflat[rows_a].astype(BF16).T
        dc = dist[sl]
        ft = np.zeros((D + 2, NPC), np.float32)
        for dd in range(D):
            ft[dd] = dc >= (1 << dd)
        ft[D] = dc.astype(np.float32) / L
        ft[D + 1] = 1.0  # carries b_in through wc_ext
        xin[: D + 2, 2, :NPC] = ft.astype(BF16)
        in_maps.append({"xin": xin, **wmap})
    return in_maps


def _numpy_fallback(
    x, xxj, W_donor, W_acceptor, w_dist, W_distembed, b_in, W_out, b_out
):
    bi, di, ai = xxj[:, 0], xxj[:, 1], xxj[:, 2]
    out = np.empty((N, T), np.float32)
    for s0 in range(0, N, 20000):
        s = slice(s0, min(s0 + 20000, N))
        xd = x[bi[s], di[s], :]
        xa = x[bi[s], ai[s], :]
        dist = np.abs(ai[s].astype(np.int64) - di[s].astype(np.int64))
        de = (dist[:, None] >= (1 << np.arange(D))[None, :]).astype(np.float32)
        dn = dist.astype(np.float32) / L
        y = (xd @ W_donor + xa @ W_acceptor + dn[:, None] * w_dist[None, :]
             + de @ W_distembed + b_in[None, :])
        c = np.float32(np.sqrt(2.0 / np.pi))
        y = 0.5 * y * (1.0 + np.tanh(c * (y + np.float32(0.044715) * y ** 3)))
        z = y @ W_out + b_out[None, :]
        out[s] = np.log1p(np.exp(-np.abs(z))) + np.maximum(z, 0.0)
    return out


def kernel(
    x, xxj_sparse, W_donor, W_acceptor, w_dist, W_distembed, b_in, W_out, b_out
):
    global LAST_RESULTS, USED_FALLBACK
    from concourse.bass_utils import run_bass_kernel_spmd

    x = np.asarray(x)
    xxj = np.asarray(xxj_sparse)
    if np.any(np.asarray(b_out)):
        print("kernel: nonzero b_out not folded on device; using numpy path")
        USED_FALLBACK = True
        return _numpy_fallback(
            x, xxj, W_donor, W_acceptor, w_dist, W_distembed, b_in, W_out, b_out
        )
    in_maps = _prep_inputs(
        x, xxj, W_donor, W_acceptor, w_dist, W_distembed, b_in, W_out, b_out
    )

    try:
        nc = _build()
        res = run_bass_kernel_spmd(
            nc, in_maps, core_ids=list(range(NCORES)),
            trace=bool(int(os.environ.get("KERNEL_TRACE", "0"))),
        )
    except Exception:
        import traceback

        traceback.print_exc()
        USED_FALLBACK = True
        return _numpy_fallback(
            x, xxj, W_donor, W_acceptor, w_dist, W_distembed, b_in, W_out, b_out
        )
    LAST_RESULTS = res
    USED_FALLBACK = False

    out = np.empty((N, T), np.float32)
    for c, r in enumerate(res.results):
        ob = r["outb"]  # [128, ZCOLS] = [128, NT, 4, 32] blocked
        core = ob.reshape(128, NT, 4, T).transpose(1, 2, 0, 3).reshape(NPCP, T)
        out[c * NPC : (c + 1) * NPC] = core[:NPC]
    return out


# revision 18
# speedup vs baseline: 1.1776x; 1.0050x over previous
"""Trainium2 Bass kernel for nn_JunctionCountsModel (gnn_message_passing).

out[n] = softplus(gelu(x[b,d] @ Wd + x[b,a] @ Wa + feat[n] @ Wc + b_in) @ Wo + b_out)

Sharding: data-parallel over the N=200000 junctions across 8 NeuronCores
(25000 each, padded to 25088 = 49*512); x and the MLP weights replicated.

The runtime in this container rejects the SWDGE custom-ucode DMA ops
(dma_gather / dma_scatter_add fail with INTERNAL at NEFF execution), and the
surviving gather primitives are too slow for 50k rows/core (indirect_dma_start
~1.5us/128 rows of Pool-SEQ overhead; ap_gather scans its whole SBUF table per
call).  So the donor/acceptor row gather runs on the host (numpy fancy
indexing over the known index tensor, exactly the prep step that previously
built the dma_gather index tables), staged per core as a [128, 3, NPCP] bf16
tensor in (K-partitions, donor|acceptor|feat, junction) layout — the exact
operand layout the tensor engine wants, so the device runs a pure streaming
MLP.

Device-side structure per 512-junction tile:
  - input layer: per 3-h-block group, 9 matmuls accumulate
    Wd^T xd + Wa^T xa + Wc_ext^T ft into a 3-bank PSUM tile; Wc_ext carries
    [W_distembed; w_dist; b_in] with ft rows [thresholds; dist/L; ones], so
    the bias is part of the matmul and one merged gelu activation covers all
    3 h-blocks (fewer, larger ACT instructions).  The feat operand is
    ZERO-PADDED to contract=128: a contract-17 matmul starves the PE
    activity monitor (HAM) and locks the whole stream at the cold 1.2 GHz
    clock (measured 2.07x slowdown); full-height operands keep it at 2.4.
  - output layer: junction-major — z^T[128j, 32] = sum_hb yg[hb]^T @ Wo[hb],
    4 sub-tiles per tile accumulated in a single PSUM bank, copied into an
    SBUF-resident z buffer.  No DRAM round-trip for z.
  - phase 2 (after all gelus, so the ACT table swaps exactly once):
    out = ln(1 + exp(z)) over [128, big] slabs, DMA'd out in the blocked
    layout; the host unshuffles.
"""

import os
import sys

sys.path.insert(0, "/opt/trn_rl_repo")

import ml_dtypes
import numpy as np

B, L, K = 4, 32768, 128
N, H, D, T = 200000, 768, 15, 32
NCORES = 8
NPC = N // NCORES  # junctions per core
NPCP = 25088  # padded to 49 * 512
HB = H // 128  # h-blocks
HG = 2  # activation groups of 3 h-blocks
TILE = 512
NT = NPCP // TILE  # 49 tiles
CHUNK = 2048  # junctions per input DMA chunk
ZCOLS = NPCP // TILE * 128  # z buffer free cols per partition (49*128=6272)
FCH = 4  # phase-2 chunks

BF16 = ml_dtypes.bfloat16

# Results of the most recent device run (for test harness introspection).
LAST_RESULTS = None
USED_FALLBACK = False
_BUILD_CACHE = {}


def _build():
    """Build the per-core Bass program (identical across the 8 cores)."""
    if "nc" in _BUILD_CACHE:
        return _BUILD_CACHE["nc"]
    import bass_rust
    import concourse.bacc as bacc
    import concourse.mybir as mybir
    import concourse.tile as tile

    dt = mybir.dt
    AF = mybir.ActivationFunctionType
    gelu_fn = AF.Tanh if os.environ.get("KERNEL_SIM_ACT") else AF.Gelu_apprx_tanh

    nc = bacc.Bacc("TRN2", target_bir_lowering=False, debug=False)
    xin = nc.dram_tensor("xin", [128, 3, NPCP], dt.bfloat16, kind="ExternalInput")
    wd = nc.dram_tensor("wd", [K, H], dt.bfloat16, kind="ExternalInput")
    wa = nc.dram_tensor("wa", [K, H], dt.bfloat16, kind="ExternalInput")
    wc = nc.dram_tensor("wc", [K, H], dt.bfloat16, kind="ExternalInput")
    wo = nc.dram_tensor("wo", [128, HB, T], dt.bfloat16, kind="ExternalInput")
    outb = nc.dram_tensor("outb", [128, ZCOLS], dt.float32, kind="ExternalOutput")

    with tile.TileContext(nc) as tc:
        with (
            tc.tile_pool(name="const", bufs=1) as const,
            tc.tile_pool(name="xp", bufs=3) as xp,
            tc.tile_pool(name="ygp", bufs=2) as ygp,
            tc.tile_pool(name="finp", bufs=4) as finp,
            tc.tile_pool(name="ypsum", bufs=2, space="PSUM") as ypsum,
            tc.tile_pool(name="opsum", bufs=2, space="PSUM") as opsum,
        ):
            wd_sb = const.tile([K, H], dt.bfloat16, tag="wd")
            wa_sb = const.tile([K, H], dt.bfloat16, tag="wa")
            wc_sb = const.tile([K, H], dt.bfloat16, tag="wc")
            wo_sb = const.tile([128, HB, T], dt.bfloat16, tag="wo")
            zbuf = const.tile([128, ZCOLS], dt.float32, tag="zbuf")
            nc.sync.dma_start(out=wd_sb[:], in_=wd[:])
            nc.sync.dma_start(out=wa_sb[:], in_=wa[:])
            nc.sync.dma_start(out=wc_sb[:], in_=wc[:])
            nc.sync.dma_start(out=wo_sb[:], in_=wo[:])

            def emit_output(yg, ti):
                """Output layer for a finished tile: z^T[128j, T] into zbuf."""
                zps = opsum.tile([128, 4, T], dt.float32, tag="zps")
                for s in range(4):
                    for hb in range(HB):
                        nc.tensor.matmul(
                            zps[:, s, :],
                            yg[:, hb, s * 128 : (s + 1) * 128],
                            wo_sb[:, hb, :],
                            start=(hb == 0), stop=(hb == HB - 1),
                        )
                nc.vector.tensor_copy(zbuf[:, ti * 128 : (ti + 1) * 128], zps[:])

            # Software-pipelined main loop: tile t's output matmuls are
            # emitted after tile t+1's input matmuls, so the tensor engine
            # never waits on the gelu and stays continuously busy (p-state
            # ramps to full clock).
            last_act = None
            pending = None  # (yg, tile_index) awaiting its output layer
            for c0 in range(0, NPCP, CHUNK):
                ch = min(CHUNK, NPCP - c0)
                xt = xp.tile([128, 3, CHUNK], dt.bfloat16, tag="xt")
                nc.sync.dma_start(out=xt[:, :, :ch], in_=xin[:, :, c0 : c0 + ch])
                for t0 in range(0, ch, TILE):
                    j0 = c0 + t0  # global junction column of this tile
                    yg = ygp.tile([128, HB, TILE], dt.bfloat16, tag="yg")
                    for g in range(HG):
                        yps = ypsum.tile([128, 3, TILE], dt.float32, tag="yps")
                        for i in range(3):
                            hb = 3 * g + i
                            hsl = slice(hb * 128, (hb + 1) * 128)
                            nc.tensor.matmul(
                                yps[:, i, :], wd_sb[:, hsl],
                                xt[:, 0, t0 : t0 + TILE],
                                start=True, stop=False,
                            )
                            nc.tensor.matmul(
                                yps[:, i, :], wa_sb[:, hsl],
                                xt[:, 1, t0 : t0 + TILE],
                                start=False, stop=False,
                            )
                            nc.tensor.matmul(
                                yps[:, i, :], wc_sb[:, hsl],
                                xt[:, 2, t0 : t0 + TILE],
                                start=False, stop=True,
                            )
                        last_act = nc.scalar.activation(
                            yg[:, 3 * g : 3 * g + 3, :], yps[:], gelu_fn
                        )
                    if pending is not None:
                        emit_output(*pending)
                    pending = (yg, j0 // TILE)
            emit_output(*pending)

            # phase 2: out = ln(1 + exp(z)); gated behind the last gelu so the
            # ACT table swaps exactly once
            AFexp = mybir.ActivationFunctionType.Exp
            AFln = mybir.ActivationFunctionType.Ln
            fch = ZCOLS // FCH
            # all Exp chunks first, then all Ln chunks: exactly two ACT table
            # loads (the greedy table picker would otherwise thrash
            # exp_and_others <-> natural_log per chunk)
            ebuf = const.tile([128, ZCOLS], dt.float32, tag="ebuf")
            last_exp = None
            for q in range(FCH):
                sl = slice(q * fch, (q + 1) * fch)
                ea = nc.scalar.activation(ebuf[:, sl], zbuf[:, sl], AFexp)
                bass_rust.add_dep_helper(
                    ea.ins, last_act.ins, True,
                    "exp phase strictly after all gelus",
                )
                last_exp = ea
            for q in range(FCH):
                sl = slice(q * fch, (q + 1) * fch)
                ot = finp.tile([128, fch], dt.float32, tag="ot")
                ol = nc.scalar.activation(ot[:], ebuf[:, sl], AFln, bias=1.0)
                bass_rust.add_dep_helper(
                    ol.ins, last_exp.ins, True,
                    "ln phase strictly after all exps",
                )
                nc.sync.dma_start(out=outb[:, sl], in_=ot[:])
    nc.compile()
    _BUILD_CACHE["nc"] = nc
    return nc


def _prep_inputs(x, xxj, W_donor, W_acceptor, w_dist, W_distembed, b_in, W_out, b_out):
    """Host prep: gather donor/acceptor rows, build features, pack weights."""
    xflat = np.asarray(x, np.float32).reshape(B * L, K)
    bi = xxj[:, 0].astype(np.int64)
    di = xxj[:, 1].astype(np.int64)
    ai = xxj[:, 2].astype(np.int64)
    dist = np.abs(ai - di)

    wc_ext = np.zeros((K, H), np.float32)
    wc_ext[:D] = np.asarray(W_distembed, np.float32)
    wc_ext[D] = np.asarray(w_dist, np.float32)
    wc_ext[D + 1] = np.asarray(b_in, np.float32)
    wmap = {
        "wd": np.asarray(W_donor).astype(BF16),
        "wa": np.asarray(W_acceptor).astype(BF16),
        "wc": wc_ext.astype(BF16),
        "wo": np.ascontiguousarray(
            np.asarray(W_out).astype(BF16).reshape(HB, 128, T).transpose(1, 0, 2)
        ),
    }

    in_maps = []
    for c in range(NCORES):
        sl = slice(c * NPC, (c + 1) * NPC)
        xin = np.zeros((128, 3, NPCP), BF16)
        rows_d = bi[sl] * L + di[sl]
        rows_a = bi[sl] * L + ai[sl]
        xin[:, 0, :NPC] = xflat[rows_d].astype(BF16).T
        xin[:, 1, :NPC] = xflat[rows_a].astype(BF16).T
        dc = dist[sl]
        ft = np.zeros((D + 2, NPC), np.float32)
        for dd in range(D):
            ft[dd] = dc >= (1 << dd)
        ft[D] = dc.astype(np.float32) / L
        ft[D + 1] = 1.0  # carries b_in through wc_ext
        xin[: D + 2, 2, :NPC] = ft.astype(BF16)
        in_maps.append({"xin": xin, **wmap})
    return in_maps


def _numpy_fallback(
    x, xxj, W_donor, W_acceptor, w_dist, W_distembed, b_in, W_out, b_out
):
    bi, di, ai = xxj[:, 0], xxj[:, 1], xxj[:, 2]
    out = np.empty((N, T), np.float32)
    for s0 in range(0, N, 20000):
        s = slice(s0, min(s0 + 20000, N))
        xd = x[bi[s], di[s], :]
        xa = x[bi[s], ai[s], :]
        dist = np.abs(ai[s].astype(np.int64) - di[s].astype(np.int64))
        de = (dist[:, None] >= (1 << np.arange(D))[None, :]).astype(np.float32)
        dn = dist.astype(np.float32) / L
        y = (xd @ W_donor + xa @ W_acceptor + dn[:, None] * w_dist[None, :]
             + de @ W_distembed + b_in[None, :])
        c = np.float32(np.sqrt(2.0 / np.pi))
        y = 0.5 * y * (1.0 + np.tanh(c * (y + np.float32(0.044715) * y ** 3)))
        z = y @ W_out + b_out[None, :]
        out[s] = np.log1p(np.exp(-np.abs(z))) + np.maximum(z, 0.0)
    return out


def kernel(
    x, xxj_sparse, W_donor, W_acceptor, w_dist, W_distembed, b_in, W_out, b_out
):
    global LAST_RESULTS, USED_FALLBACK
    from concourse.bass_utils import run_bass_kernel_spmd

    x = np.asarray(x)
    xxj = np.asarray(xxj_sparse)
    if np.any(np.asarray(b_out)):
        print("kernel: nonzero b_out not folded on device; using numpy path")
        USED_FALLBACK = True
        return _numpy_fallback(
            x, xxj, W_donor, W_acceptor, w_dist, W_distembed, b_in, W_out, b_out
        )
    in_maps = _prep_inputs(
        x, xxj, W_donor, W_acceptor, w_dist, W_distembed, b_in, W_out, b_out
    )

    try:
        nc = _build()
        res = run_bass_kernel_spmd(
            nc, in_maps, core_ids=list(range(NCORES)),
            trace=bool(int(os.environ.get("KERNEL_TRACE", "0"))),
        )
    except Exception:
        import traceback

        traceback.print_exc()
        USED_FALLBACK = True
        return _numpy_fallback(
            x, xxj, W_donor, W_acceptor, w_dist, W_distembed, b_in, W_out, b_out
        )
    LAST_RESULTS = res
    USED_FALLBACK = False

    out = np.empty((N, T), np.float32)
    for c, r in enumerate(res.results):
        ob = r["outb"]  # [128, ZCOLS] = [128, NT, 4, 32] blocked
        core = ob.reshape(128, NT, 4, T).transpose(1, 2, 0, 3).reshape(NPCP, T)
        out[c * NPC : (c + 1) * NPC] = core[:NPC]
    return out
